# revision 9
# baseline (speedup 1.0000x reference)
"""Trainium2 Bass kernel for nn_EpisodicMemoryEfficient.

Computation (see reference): per batch, x = states reshaped [S, 512];
q/k/v = x @ {Wq,Wk,Wv}.T; chunked sliding-window attention with chunk size
C=64 where chunk i attends to chunks {i-1, i} with strict causal masking
(chunk 0 has no previous chunk), softmax over the 128-key context, out = p@v.

Sharding: pure data parallel over (batch, sequence-half) -> 8 cores.
Each core receives a pre-transposed xT [512, 4224] covering its 4096-row
shard plus a 128-row halo in front (zeros for the first half, the real
previous rows for the second half).

Device kernel layout choices (driven by the TRN2 cost model: matmul cost =
moving-free-dim x 0.417ns x cycles/row; bf16 is 1 cycle/row at ANY width,
fp8e4 with DoubleRow packs two 128-row contraction subtiles per matmul):
  - the k projection is fused away algebraically: scores = q.k^T =
    x (Wq^T Wk) x^T, with M = Wq^T Wk precomputed on the host. The xT
    tiles already resident in SBUF serve directly as the stationary
    operand of the score matmuls, so only TWO 512x512 projections run on
    device (q'' = x@M, transposed; v = x@Wv^T, natural).
  - both projections run in fp8e4m3 DoubleRow mode (half the matmul
    count); the weights are scaled x16 on the host so their ~0.05-rms
    entries sit in e4m3's normal range. The x16 on q'' is undone inside
    the exp scale; the x16 on v is undone by setting the
    denominator-matmul ones vector to 16.
  - score and PV matmuls use bf16 operands (PV contracts only 128 rows
    per region, so DoubleRow could not help it on hardware).
  - attention uses transposed scores sT[j, qi] per single 128-row q-tile
    (region B = own x-tile keys [j 128, qi 128]; region A = the previous
    x-tile's second chunk vs first-chunk queries, computed as a [64, 64]
    quadrant). The softmax denominator is a ~free N=1 bf16 matmul and PV
    needs no transpose (lhsT = masked exp, rhs = natural v).
  - masking is a 0/1 bf16 multiply AFTER exp (exp reads raw scores
    directly from PSUM; the region-A quadrant needs no mask at all except
    for the very first chunk, handled by a per-core mask variant).
  - attention is software-pipelined two units deep so the in-order PE
    never stalls on the ACT-exp -> DVE-mask chain.
The only fully-masked row (global position 0) yields 0/0 -> NaN and is
overwritten with zeros on the host, matching the reference.
"""

import numpy as np

_D = 512
_C = 64
_HALO = 128  # two chunks of halo keeps everything 128-aligned
_SCALE = 1.0 / float(np.sqrt(_D))
_W8SCALE = 16.0  # fp8 weight pre-scale (power of two; undone downstream)

_BUILD_CACHE = {}


def _build(L, repeat=1, aquad=True, halo_skip=True, lag=2, fp8_proj=False,
           out_bf16=True):
    """Build the per-core Bass kernel for a shard of L query rows.

    repeat>1 wraps the whole body in a For_i loop — used only for
    wall-clock timing (amortizes the ~100ms axon dispatch overhead).
    """
    from contextlib import ExitStack

    import concourse.mybir as mybir
    import concourse.tile as tile
    from concourse import bacc

    assert L % 128 == 0 and L >= 256
    NX = L + _HALO          # x rows held on-core (halo + shard)
    NQ = L // 128           # q-tiles
    dt = mybir.dt.float32
    dtb = mybir.dt.bfloat16
    dt8 = mybir.dt.float8e4
    AW = 64 if aquad else 128   # region-A score width (qi) and j extent
    SW = 128 + AW               # score tile free size

    nc = bacc.Bacc("TRN2", target_bir_lowering=False, debug=False, num_devices=8)

    xT_d = nc.dram_tensor("xT", [_D, NX], dtb, kind="ExternalInput")
    if fp8_proj:
        x8_d = nc.dram_tensor("x8", [256, 2, NX], dt8, kind="ExternalInput")
        mw_d = nc.dram_tensor("mw", [256, 2, _D], dt8, kind="ExternalInput")
        wvt_d = nc.dram_tensor("wvt", [256, 2, _D], dt8, kind="ExternalInput")
    else:
        mw_d = nc.dram_tensor("mw", [_D, _D], dtb, kind="ExternalInput")
        wvt_d = nc.dram_tensor("wvt", [_D, _D], dtb, kind="ExternalInput")
    m01_d = nc.dram_tensor("m01", [128, SW], dtb, kind="ExternalInput")
    m01z_d = nc.dram_tensor("m01z", [128, SW], dtb, kind="ExternalInput")
    out_d = nc.dram_tensor("out", [L, _D], dtb if out_bf16 else dt,
                           kind="ExternalOutput")

    # x column blocks: [512 x 8, 128]; blocks start at even tile indices.
    blocks = []
    m0 = 0
    while m0 < NX:
        mb = min(512, NX - m0)
        blocks.append((m0, mb))
        m0 += mb
    blk_of_tile = {}
    for bi, (m0, mb) in enumerate(blocks):
        for s in range(mb // 128):
            blk_of_tile[m0 // 128 + s] = (bi, s)

    with ExitStack() as ctx:
        tc = ctx.enter_context(tile.TileContext(nc))
        const = ctx.enter_context(tc.tile_pool(name="const", bufs=1))
        xpool = ctx.enter_context(tc.tile_pool(name="xpool", bufs=3))
        qpool = ctx.enter_context(tc.tile_pool(name="qpool", bufs=3))
        vpool = ctx.enter_context(tc.tile_pool(name="vpool", bufs=4))
        spool = ctx.enter_context(tc.tile_pool(name="spool", bufs=4))
        opool = ctx.enter_context(tc.tile_pool(name="opool", bufs=4))
        ps_proj = ctx.enter_context(tc.tile_pool(name="ps_proj", bufs=2, space="PSUM"))
        ps_att = ctx.enter_context(tc.tile_pool(name="ps_att", bufs=2, space="PSUM"))
        ps_out = ctx.enter_context(tc.tile_pool(name="ps_out", bufs=2, space="PSUM"))
        ps_sum = ctx.enter_context(tc.tile_pool(name="ps_sum", bufs=2, space="PSUM"))

        # ---- constants (DMAs emitted inside body() after block 0's x loads
        # so the first projection matmuls aren't head-blocked on the weight
        # traffic) ----
        if fp8_proj:
            mw_sb = [const.tile([128, 2, _D], dt8, tag=f"mw{p}", name=f"mw{p}")
                     for p in range(2)]
            wv_sb = [const.tile([128, 2, _D], dt8, tag=f"wv{p}", name=f"wv{p}")
                     for p in range(2)]
        else:
            mw_sb = [const.tile([128, _D], dtb, tag=f"mw{c}", name=f"mw{c}")
                     for c in range(4)]
            wv_sb = [const.tile([128, _D], dtb, tag=f"wv{c}", name=f"wv{c}")
                     for c in range(4)]
        m01_sb = const.tile([128, SW], dtb, tag="m01", name="m01_sb")
        m01z_sb = const.tile([128, SW], dtb, tag="m01z", name="m01z_sb")
        ones_sb = const.tile([128, 1], dtb, tag="ones", name="ones_sb")

        def load_consts():
            if fp8_proj:
                for p in range(2):
                    nc.sync.dma_start(out=mw_sb[p], in_=mw_d[128 * p:128 * (p + 1)])
                for p in range(2):
                    nc.sync.dma_start(out=wv_sb[p], in_=wvt_d[128 * p:128 * (p + 1)])
            else:
                for c in range(4):
                    nc.sync.dma_start(out=mw_sb[c], in_=mw_d[128 * c:128 * (c + 1), :])
                for c in range(4):
                    nc.sync.dma_start(out=wv_sb[c], in_=wvt_d[128 * c:128 * (c + 1), :])
            nc.sync.dma_start(out=m01_sb, in_=m01_d[:, :])
            nc.sync.dma_start(out=m01z_sb, in_=m01z_d[:, :])
            # v carries the x16 fp8 weight scale; a 16.0 in the denominator
            # ones cancels it (recip = 1/(16*rowsum); po = 16*true)
            nc.vector.memset(ones_sb, _W8SCALE if fp8_proj else 1.0)

        qt_tiles = {}   # (block, c) -> [128, *] bf16 q''T tile
        v_tiles = {}    # x-tile index -> [128, 512] natural bf16 v tile
        xt_loaded = {}  # block -> list of 4 [128, mb] bf16 xT tiles
        x8_loaded = {}  # block -> list of 2 [128, 2, mb] fp8 xT tiles

        def xt_slice(ti, c, j0=0, w=128):
            bi, s = blk_of_tile[ti]
            return xt_loaded[bi][c][:, s * 128 + j0:s * 128 + j0 + w]

        def qt_slice(ti, c, w=128):
            bi, s = blk_of_tile[ti]
            off = s * 128 - (128 if (halo_skip and bi == 0) else 0)
            return qt_tiles[(bi, c)][:, off:off + w]

        def load_x(bi):
            m0, mb = blocks[bi]
            xt = []
            for c in range(4):
                t_ = xpool.tile([128, mb], dtb, tag=f"xt{c}", name=f"xt{c}_b{bi}")
                nc.sync.dma_start(out=t_, in_=xT_d[128 * c:128 * (c + 1), m0:m0 + mb])
                xt.append(t_)
            xt_loaded[bi] = xt
            if fp8_proj:
                x8 = []
                for p in range(2):
                    t8 = xpool.tile([128, 2, mb], dt8, tag=f"x8{p}",
                                    name=f"x8{p}_b{bi}")
                    nc.sync.dma_start(out=t8,
                                      in_=x8_d[128 * p:128 * (p + 1), :, m0:m0 + mb])
                    x8.append(t8)
                x8_loaded[bi] = x8

        def project_block(bi):
            m0, mb = blocks[bi]
            if bi not in xt_loaded:
                load_x(bi)
            xt = xt_loaded[bi]
            q0 = 128 if (halo_skip and bi == 0) else 0  # skip halo q''
            qb = mb - q0
            for o in range(4):
                psq = ps_proj.tile([128, qb], dt, tag="ps_proj", name=f"psq{o}_b{bi}")
                if fp8_proj:
                    x8 = x8_loaded[bi]
                    for p in range(2):
                        nc.tensor.matmul(
                            psq, mw_sb[p][:, :, 128 * o:128 * (o + 1)],
                            x8[p][:, :, q0:q0 + qb],
                            start=(p == 0), stop=(p == 1),
                            perf_mode=mybir.MatmulPerfMode.DoubleRow)
                else:
                    for c in range(4):
                        nc.tensor.matmul(psq, mw_sb[c][:, 128 * o:128 * (o + 1)],
                                         xt[c][:, q0:q0 + qb],
                                         start=(c == 0), stop=(c == 3))
                qt = qpool.tile([128, qb], dtb, tag=f"qt{o}", name=f"qt{o}_b{bi}")
                nc.vector.tensor_copy(qt, psq)
                qt_tiles[(bi, o)] = qt
            for s in range(mb // 128):
                ti = m0 // 128 + s
                psv = ps_proj.tile([128, _D], dt, tag="ps_proj", name=f"psv{s}_b{bi}")
                if fp8_proj:
                    x8 = x8_loaded[bi]
                    for p in range(2):
                        nc.tensor.matmul(
                            psv, x8[p][:, :, 128 * s:128 * (s + 1)], wv_sb[p],
                            start=(p == 0), stop=(p == 1),
                            perf_mode=mybir.MatmulPerfMode.DoubleRow)
                else:
                    for c in range(4):
                        nc.tensor.matmul(psv, xt[c][:, 128 * s:128 * (s + 1)],
                                         wv_sb[c], start=(c == 0), stop=(c == 3))
                vt = vpool.tile([128, _D], dtb, tag=f"v{ti % 4}", name=f"v_t{ti}")
                nc.scalar.copy(vt, psv)
                v_tiles[ti] = vt

        def attend_scores(t):
            """q = x-tile t+1; keys region B = x-tile t+1 (cols 0:128),
            region A = x-tile t (cols 128:SW). With aquad, region A is only
            the fully-allowed [j 64:128, qi 0:64] quadrant, landing on PSUM
            partitions 0:64 (rows 64:128 of its columns are stale PSUM the
            mask multiply zeroes or NaNs — never consumed by the PV/sum
            matmuls, which read partitions 0:64 only)."""
            ps_s = ps_att.tile([128, SW], dt, tag="ps_s", name=f"ps_s_t{t}")
            for c in range(4):
                nc.tensor.matmul(ps_s[:, 0:128], xt_slice(t + 1, c),
                                 qt_slice(t + 1, c),
                                 start=(c == 0), stop=(c == 3))
            j0 = 128 - AW  # aquad: only the previous tile's second chunk;
            # its scores land on PSUM partitions j0:128 so the PV matmul's
            # lhsT shares v_tiles[t][j0:128]'s base partition
            for c in range(4):
                nc.tensor.matmul(ps_s[j0:128, 128:SW], xt_slice(t, c, j0=j0, w=AW),
                                 qt_slice(t + 1, c, w=AW),
                                 start=(c == 0), stop=(c == 3))
            e_sb = spool.tile([128, SW], dtb, tag="e_sb", name=f"e_t{t}")
            nc.scalar.activation(e_sb, ps_s, mybir.ActivationFunctionType.Exp,
                                 scale=_SCALE / (_W8SCALE if fp8_proj else 1.0))
            em = spool.tile([128, SW], dtb, tag="em", name=f"em_t{t}")
            nc.vector.tensor_mul(em, e_sb, m01z_sb if t == 0 else m01_sb)
            return em

        def attend_finish(t, em):
            """softmax denominator + PV + normalize + store for q-tile t."""
            psum_t = ps_sum.tile([128, 1], dt, tag="ps_sum", name=f"pssum_t{t}")
            po = ps_out.tile([128, _D], dt, tag="ps_o", name=f"ps_o_t{t}")
            # region B first (writes all 128 partitions with start=True);
            # the region-A quadrant then accumulates into partitions 0:AW
            nc.tensor.matmul(psum_t, em[:, 0:128], ones_sb,
                             start=True, stop=(AW == 0))
            nc.tensor.matmul(po, em[:, 0:128], v_tiles[t + 1],
                             start=True, stop=(AW == 0))
            j0 = 128 - AW
            nc.tensor.matmul(psum_t[0:AW], em[j0:128, 128:SW], ones_sb[j0:128],
                             start=False, stop=True)
            nc.tensor.matmul(po[0:AW], em[j0:128, 128:SW],
                             v_tiles[t][j0:128, :], start=False, stop=True)
            recip = opool.tile([128, 1], dt, tag="recip", name=f"recip_t{t}")
            nc.vector.reciprocal(recip, psum_t)
            o_sb = opool.tile([128, _D], dtb if out_bf16 else dt, tag="o_sb",
                              name=f"o_t{t}")
            nc.scalar.mul(o_sb, po, recip)
            nc.sync.dma_start(out=out_d[128 * t:128 * (t + 1), :], in_=o_sb)

        def body():
            xt_loaded.clear()
            x8_loaded.clear()
            qt_tiles.clear()
            v_tiles.clear()
            load_x(0)
            load_consts()
            pending = []  # software pipeline: [(t, em)]
            for bi in range(len(blocks)):
                project_block(bi)
                # attention units whose q x-tile lives in block bi
                for t in range(NQ):
                    if blk_of_tile[t + 1][0] != bi:
                        continue
                    em = attend_scores(t)
                    pending.append((t, em))
                    if len(pending) > lag:
                        attend_finish(*pending.pop(0))
            while pending:
                attend_finish(*pending.pop(0))

        if repeat == 1:
            body()
        else:
            with tc.For_i(0, repeat, 1,
                          hint_engines=(mybir.EngineType.PE, mybir.EngineType.DVE,
                                        mybir.EngineType.Activation,
                                        mybir.EngineType.SP)):
                body()

    nc.compile()
    return nc


def _get_nc(L):
    if L not in _BUILD_CACHE:
        _BUILD_CACHE[L] = _build(L)
    return _BUILD_CACHE[L]


def _masks(aquad=True):
    """0/1 single-tile masks [j 128, qi 128 | A] in the transposed-scores
    orientation. Region B (cols 0:128): keys in the query's own x-tile —
    strict causal p < qi. Region A: the preceding x-tile — only its second
    chunk and only for the first-chunk queries (qi < 64). With aquad the
    region-A block is the [64, 64] fully-allowed quadrant (mask = 1 on
    partitions 0:64). m01z is the no-previous-chunk variant for global
    chunk 0 (region A fully zero)."""
    import ml_dtypes

    p = np.arange(128)[:, None]
    q = np.arange(128)[None, :]
    mb_ = (p < q).astype(np.float32)
    if aquad:
        ma_ = np.zeros((128, 64), np.float32)
        ma_[64:128, :] = 1.0
    else:
        ma_ = ((p >= 64) & (q < 64)).astype(np.float32)
    m01 = np.concatenate([mb_, ma_], axis=1).astype(ml_dtypes.bfloat16)
    m01z = np.concatenate([mb_, np.zeros_like(ma_)],
                          axis=1).astype(ml_dtypes.bfloat16)
    return m01, m01z


def _make_in_maps(states, Wq, Wk, Wv, aquad=True, fp8_proj=False):
    import ml_dtypes

    bf16 = ml_dtypes.bfloat16
    f8 = ml_dtypes.float8_e4m3
    states = np.ascontiguousarray(np.asarray(states, dtype=np.float32))
    B, S = states.shape[0], states.shape[1]
    x = states.reshape(B, S, _D)
    L = S // 2
    NX = L + _HALO
    Wq = np.asarray(Wq, dtype=np.float32)
    Wk = np.asarray(Wk, dtype=np.float32)
    Wv = np.asarray(Wv, dtype=np.float32)
    # scores = q.k^T = x (Wq^T Wk) x^T — fold both weights into one matrix
    mw_f = np.ascontiguousarray(Wq.T @ Wk)
    wvt_f = np.ascontiguousarray(Wv.T)
    if fp8_proj:
        mw = (mw_f * _W8SCALE).astype(f8).reshape(256, 2, _D)
        wvt = (wvt_f * _W8SCALE).astype(f8).reshape(256, 2, _D)
    else:
        mw = mw_f.astype(bf16)
        wvt = wvt_f.astype(bf16)
    m01, m01z = _masks(aquad)
    in_maps = []
    for core in range(2 * B):
        b, h = core // 2, core % 2
        xp = np.zeros((NX, _D), dtype=np.float32)
        lo = h * L - _HALO
        if lo < 0:
            xp[_HALO:] = x[b, 0:L]
        else:
            xp[:] = x[b, lo:lo + NX]
        xT = np.ascontiguousarray(xp.T)
        m = {
            "xT": xT.astype(bf16),
            "mw": mw, "wvt": wvt,
            "m01": m01, "m01z": m01z if h == 0 else m01,
        }
        if fp8_proj:
            m["x8"] = xT.astype(f8).reshape(256, 2, NX)
        in_maps.append(m)
    return in_maps


def _assemble(results, B, S):
    L = S // 2
    out = np.empty((B, S, _D), dtype=np.float32)
    for core, res in enumerate(results):
        b, h = core // 2, core % 2
        out[b, h * L:(h + 1) * L] = res["out"].astype(np.float32)
    out[:, 0, :] = 0.0
    return out.reshape(B, S, _D // 2, 2)


def run(states, Wq, Wk, Wv, trace=False):
    """Run on 8 NeuronCores; returns (output, BassKernelResults)."""
    from concourse.bass_utils import run_bass_kernel_spmd

    states = np.asarray(states)
    B, S = states.shape[0], states.shape[1]
    assert B == 4 and S % 128 == 0
    nc = _get_nc(S // 2)
    in_maps = _make_in_maps(states, Wq, Wk, Wv)
    try:
        res = run_bass_kernel_spmd(nc, in_maps, core_ids=list(range(8)), trace=trace)
    except ModuleNotFoundError:
        # axon NTFF hook unavailable in this container — run untraced
        res = run_bass_kernel_spmd(nc, in_maps, core_ids=list(range(8)), trace=False)
    return _assemble(res.results, B, S), res


def kernel(states, Wq, Wk, Wv):
    out, _ = run(states, Wq, Wk, Wv, trace=False)
    return out


# revision 10
# speedup vs baseline: 1.1102x; 1.1102x over previous
"""Trainium2 Bass kernel for nn_EpisodicMemoryEfficient.

Computation (see reference): per batch, x = states reshaped [S, 512];
q/k/v = x @ {Wq,Wk,Wv}.T; chunked sliding-window attention with chunk size
C=64 where chunk i attends to chunks {i-1, i} with strict causal masking
(chunk 0 has no previous chunk), softmax over the 128-key context, out = p@v.

Sharding: pure data parallel over (batch, sequence-half) -> 8 cores.
Each core receives a pre-transposed xT [512, 4224] covering its 4096-row
shard plus a 128-row halo in front (zeros for the first half, the real
previous rows for the second half).

Device kernel layout choices (driven by the TRN2 cost model: matmul cost =
moving-free-dim x 0.417ns x cycles/row; bf16 is 1 cycle/row at ANY width,
fp8e4 with DoubleRow packs two 128-row contraction subtiles per matmul):
  - the k projection is fused away algebraically: scores = q.k^T =
    x (Wq^T Wk) x^T, with M = Wq^T Wk precomputed on the host. The xT
    tiles already resident in SBUF serve directly as the stationary
    operand of the score matmuls, so only TWO 512x512 projections run on
    device (q'' = x@M, transposed; v = x@Wv^T, natural).
  - both projections run in fp8e4m3 DoubleRow mode (half the matmul
    count); the weights are scaled x16 on the host so their ~0.05-rms
    entries sit in e4m3's normal range. The x16 on q'' is undone inside
    the exp scale; the x16 on v is undone by setting the
    denominator-matmul ones vector to 16.
  - score and PV matmuls use bf16 operands (PV contracts only 128 rows
    per region, so DoubleRow could not help it on hardware).
  - attention uses transposed scores sT[j, qi] per single 128-row q-tile
    (region B = own x-tile keys [j 128, qi 128]; region A = the previous
    x-tile's second chunk vs first-chunk queries, computed as a [64, 64]
    quadrant). The softmax denominator is a ~free N=1 bf16 matmul and PV
    needs no transpose (lhsT = masked exp, rhs = natural v).
  - masking is a 0/1 bf16 multiply AFTER exp (exp reads raw scores
    directly from PSUM; the region-A quadrant needs no mask at all except
    for the very first chunk, handled by a per-core mask variant).
  - attention is software-pipelined two units deep so the in-order PE
    never stalls on the ACT-exp -> DVE-mask chain.
The only fully-masked row (global position 0) yields 0/0 -> NaN and is
overwritten with zeros on the host, matching the reference.
"""

import numpy as np

_D = 512
_C = 64
_HALO = 128  # two chunks of halo keeps everything 128-aligned
_SCALE = 1.0 / float(np.sqrt(_D))
_W8SCALE = 16.0  # fp8 weight pre-scale (power of two; undone downstream)

_BUILD_CACHE = {}


def _build(L, repeat=1, aquad=False, halo_skip=True, lag=2, fp8_proj=False,
           out_bf16=True):
    """Build the per-core Bass kernel for a shard of L query rows.

    repeat>1 wraps the whole body in a For_i loop — used only for
    wall-clock timing (amortizes the ~100ms axon dispatch overhead).
    """
    from contextlib import ExitStack

    import concourse.mybir as mybir
    import concourse.tile as tile
    from concourse import bacc

    assert L % 128 == 0 and L >= 256
    NX = L + _HALO          # x rows held on-core (halo + shard)
    NQ = L // 128           # q-tiles
    dt = mybir.dt.float32
    dtb = mybir.dt.bfloat16
    dt8 = mybir.dt.float8e4
    AW = 64 if aquad else 128   # region-A score width (qi) and j extent
    SW = 128 + AW               # score tile free size

    nc = bacc.Bacc("TRN2", target_bir_lowering=False, debug=False, num_devices=8)

    xT_d = nc.dram_tensor("xT", [_D, NX], dtb, kind="ExternalInput")
    if fp8_proj:
        x8_d = nc.dram_tensor("x8", [256, 2, NX], dt8, kind="ExternalInput")
        mw_d = nc.dram_tensor("mw", [256, 2, _D], dt8, kind="ExternalInput")
        wvt_d = nc.dram_tensor("wvt", [256, 2, _D], dt8, kind="ExternalInput")
    else:
        mw_d = nc.dram_tensor("mw", [_D, _D], dtb, kind="ExternalInput")
        wvt_d = nc.dram_tensor("wvt", [_D, _D], dtb, kind="ExternalInput")
    m01_d = nc.dram_tensor("m01", [128, SW], dtb, kind="ExternalInput")
    m01z_d = nc.dram_tensor("m01z", [128, SW], dtb, kind="ExternalInput")
    out_d = nc.dram_tensor("out", [L, _D], dtb if out_bf16 else dt,
                           kind="ExternalOutput")

    # x column blocks: [512 x 8, 128]; blocks start at even tile indices.
    blocks = []
    m0 = 0
    while m0 < NX:
        mb = min(512, NX - m0)
        blocks.append((m0, mb))
        m0 += mb
    blk_of_tile = {}
    for bi, (m0, mb) in enumerate(blocks):
        for s in range(mb // 128):
            blk_of_tile[m0 // 128 + s] = (bi, s)

    with ExitStack() as ctx:
        tc = ctx.enter_context(tile.TileContext(nc))
        const = ctx.enter_context(tc.tile_pool(name="const", bufs=1))
        xpool = ctx.enter_context(tc.tile_pool(name="xpool", bufs=3))
        qpool = ctx.enter_context(tc.tile_pool(name="qpool", bufs=3))
        vpool = ctx.enter_context(tc.tile_pool(name="vpool", bufs=4))
        spool = ctx.enter_context(tc.tile_pool(name="spool", bufs=4))
        opool = ctx.enter_context(tc.tile_pool(name="opool", bufs=4))
        ps_proj = ctx.enter_context(tc.tile_pool(name="ps_proj", bufs=2, space="PSUM"))
        ps_att = ctx.enter_context(tc.tile_pool(name="ps_att", bufs=2, space="PSUM"))
        ps_out = ctx.enter_context(tc.tile_pool(name="ps_out", bufs=2, space="PSUM"))
        ps_sum = ctx.enter_context(tc.tile_pool(name="ps_sum", bufs=2, space="PSUM"))

        # ---- constants (DMAs emitted inside body() after block 0's x loads
        # so the first projection matmuls aren't head-blocked on the weight
        # traffic) ----
        if fp8_proj:
            mw_sb = [const.tile([128, 2, _D], dt8, tag=f"mw{p}", name=f"mw{p}")
                     for p in range(2)]
            wv_sb = [const.tile([128, 2, _D], dt8, tag=f"wv{p}", name=f"wv{p}")
                     for p in range(2)]
        else:
            mw_sb = [const.tile([128, _D], dtb, tag=f"mw{c}", name=f"mw{c}")
                     for c in range(4)]
            wv_sb = [const.tile([128, _D], dtb, tag=f"wv{c}", name=f"wv{c}")
                     for c in range(4)]
        m01_sb = const.tile([128, SW], dtb, tag="m01", name="m01_sb")
        m01z_sb = const.tile([128, SW], dtb, tag="m01z", name="m01z_sb")
        ones_sb = const.tile([128, 1], dtb, tag="ones", name="ones_sb")

        def load_consts():
            if fp8_proj:
                for p in range(2):
                    nc.sync.dma_start(out=mw_sb[p], in_=mw_d[128 * p:128 * (p + 1)])
                for p in range(2):
                    nc.sync.dma_start(out=wv_sb[p], in_=wvt_d[128 * p:128 * (p + 1)])
            else:
                for c in range(4):
                    nc.sync.dma_start(out=mw_sb[c], in_=mw_d[128 * c:128 * (c + 1), :])
                for c in range(4):
                    nc.sync.dma_start(out=wv_sb[c], in_=wvt_d[128 * c:128 * (c + 1), :])
            nc.sync.dma_start(out=m01_sb, in_=m01_d[:, :])
            nc.sync.dma_start(out=m01z_sb, in_=m01z_d[:, :])
            # v carries the x16 fp8 weight scale; a 16.0 in the denominator
            # ones cancels it (recip = 1/(16*rowsum); po = 16*true)
            nc.vector.memset(ones_sb, _W8SCALE if fp8_proj else 1.0)

        qt_tiles = {}   # (block, c) -> [128, *] bf16 q''T tile
        v_tiles = {}    # x-tile index -> [128, 512] natural bf16 v tile
        xt_loaded = {}  # block -> list of 4 [128, mb] bf16 xT tiles
        x8_loaded = {}  # block -> list of 2 [128, 2, mb] fp8 xT tiles

        def xt_slice(ti, c, j0=0, w=128):
            bi, s = blk_of_tile[ti]
            return xt_loaded[bi][c][:, s * 128 + j0:s * 128 + j0 + w]

        def qt_slice(ti, c, w=128):
            bi, s = blk_of_tile[ti]
            off = s * 128 - (128 if (halo_skip and bi == 0) else 0)
            return qt_tiles[(bi, c)][:, off:off + w]

        def load_x(bi):
            m0, mb = blocks[bi]
            xt = []
            for c in range(4):
                t_ = xpool.tile([128, mb], dtb, tag=f"xt{c}", name=f"xt{c}_b{bi}")
                nc.sync.dma_start(out=t_, in_=xT_d[128 * c:128 * (c + 1), m0:m0 + mb])
                xt.append(t_)
            xt_loaded[bi] = xt
            if fp8_proj:
                x8 = []
                for p in range(2):
                    t8 = xpool.tile([128, 2, mb], dt8, tag=f"x8{p}",
                                    name=f"x8{p}_b{bi}")
                    nc.sync.dma_start(out=t8,
                                      in_=x8_d[128 * p:128 * (p + 1), :, m0:m0 + mb])
                    x8.append(t8)
                x8_loaded[bi] = x8

        def project_block(bi):
            m0, mb = blocks[bi]
            if bi not in xt_loaded:
                load_x(bi)
            xt = xt_loaded[bi]
            q0 = 128 if (halo_skip and bi == 0) else 0  # skip halo q''
            qb = mb - q0
            for o in range(4):
                psq = ps_proj.tile([128, qb], dt, tag="ps_proj", name=f"psq{o}_b{bi}")
                if fp8_proj:
                    x8 = x8_loaded[bi]
                    for p in range(2):
                        nc.tensor.matmul(
                            psq, mw_sb[p][:, :, 128 * o:128 * (o + 1)],
                            x8[p][:, :, q0:q0 + qb],
                            start=(p == 0), stop=(p == 1),
                            perf_mode=mybir.MatmulPerfMode.DoubleRow)
                else:
                    for c in range(4):
                        nc.tensor.matmul(psq, mw_sb[c][:, 128 * o:128 * (o + 1)],
                                         xt[c][:, q0:q0 + qb],
                                         start=(c == 0), stop=(c == 3))
                qt = qpool.tile([128, qb], dtb, tag=f"qt{o}", name=f"qt{o}_b{bi}")
                nc.vector.tensor_copy(qt, psq)
                qt_tiles[(bi, o)] = qt
            for s in range(mb // 128):
                ti = m0 // 128 + s
                psv = ps_proj.tile([128, _D], dt, tag="ps_proj", name=f"psv{s}_b{bi}")
                if fp8_proj:
                    x8 = x8_loaded[bi]
                    for p in range(2):
                        nc.tensor.matmul(
                            psv, x8[p][:, :, 128 * s:128 * (s + 1)], wv_sb[p],
                            start=(p == 0), stop=(p == 1),
                            perf_mode=mybir.MatmulPerfMode.DoubleRow)
                else:
                    for c in range(4):
                        nc.tensor.matmul(psv, xt[c][:, 128 * s:128 * (s + 1)],
                                         wv_sb[c], start=(c == 0), stop=(c == 3))
                vt = vpool.tile([128, _D], dtb, tag=f"v{ti % 4}", name=f"v_t{ti}")
                nc.scalar.copy(vt, psv)
                v_tiles[ti] = vt

        def attend_scores(t):
            """q = x-tile t+1; keys region B = x-tile t+1 (cols 0:128),
            region A = x-tile t (cols 128:SW). With aquad, region A is only
            the fully-allowed [j 64:128, qi 0:64] quadrant, landing on PSUM
            partitions 0:64 (rows 64:128 of its columns are stale PSUM the
            mask multiply zeroes or NaNs — never consumed by the PV/sum
            matmuls, which read partitions 0:64 only)."""
            ps_s = ps_att.tile([128, SW], dt, tag="ps_s", name=f"ps_s_t{t}")
            for c in range(4):
                nc.tensor.matmul(ps_s[:, 0:128], xt_slice(t + 1, c),
                                 qt_slice(t + 1, c),
                                 start=(c == 0), stop=(c == 3))
            j0 = 128 - AW  # aquad: only the previous tile's second chunk;
            # its scores land on PSUM partitions j0:128 so the PV matmul's
            # lhsT shares v_tiles[t][j0:128]'s base partition
            for c in range(4):
                nc.tensor.matmul(ps_s[j0:128, 128:SW], xt_slice(t, c, j0=j0, w=AW),
                                 qt_slice(t + 1, c, w=AW),
                                 start=(c == 0), stop=(c == 3))
            e_sb = spool.tile([128, SW], dtb, tag="e_sb", name=f"e_t{t}")
            nc.scalar.activation(e_sb, ps_s, mybir.ActivationFunctionType.Exp,
                                 scale=_SCALE / (_W8SCALE if fp8_proj else 1.0))
            em = spool.tile([128, SW], dtb, tag="em", name=f"em_t{t}")
            nc.vector.tensor_mul(em, e_sb, m01z_sb if t == 0 else m01_sb)
            return em

        def attend_finish(t, em):
            """softmax denominator + PV + normalize + store for q-tile t."""
            psum_t = ps_sum.tile([128, 1], dt, tag="ps_sum", name=f"pssum_t{t}")
            po = ps_out.tile([128, _D], dt, tag="ps_o", name=f"ps_o_t{t}")
            # region B first (writes all 128 partitions with start=True);
            # the region-A quadrant then accumulates into partitions 0:AW
            nc.tensor.matmul(psum_t, em[:, 0:128], ones_sb,
                             start=True, stop=(AW == 0))
            nc.tensor.matmul(po, em[:, 0:128], v_tiles[t + 1],
                             start=True, stop=(AW == 0))
            j0 = 128 - AW
            nc.tensor.matmul(psum_t[0:AW], em[j0:128, 128:SW], ones_sb[j0:128],
                             start=False, stop=True)
            nc.tensor.matmul(po[0:AW], em[j0:128, 128:SW],
                             v_tiles[t][j0:128, :], start=False, stop=True)
            recip = opool.tile([128, 1], dt, tag="recip", name=f"recip_t{t}")
            nc.vector.reciprocal(recip, psum_t)
            o_sb = opool.tile([128, _D], dtb if out_bf16 else dt, tag="o_sb",
                              name=f"o_t{t}")
            nc.scalar.mul(o_sb, po, recip)
            nc.sync.dma_start(out=out_d[128 * t:128 * (t + 1), :], in_=o_sb)

        def body():
            xt_loaded.clear()
            x8_loaded.clear()
            qt_tiles.clear()
            v_tiles.clear()
            load_x(0)
            load_consts()
            pending = []  # software pipeline: [(t, em)]
            for bi in range(len(blocks)):
                project_block(bi)
                # attention units whose q x-tile lives in block bi
                for t in range(NQ):
                    if blk_of_tile[t + 1][0] != bi:
                        continue
                    em = attend_scores(t)
                    pending.append((t, em))
                    if len(pending) > lag:
                        attend_finish(*pending.pop(0))
            while pending:
                attend_finish(*pending.pop(0))

        if repeat == 1:
            body()
        else:
            with tc.For_i(0, repeat, 1,
                          hint_engines=(mybir.EngineType.PE, mybir.EngineType.DVE,
                                        mybir.EngineType.Activation,
                                        mybir.EngineType.SP)):
                body()

    nc.compile()
    return nc


def _get_nc(L):
    if L not in _BUILD_CACHE:
        _BUILD_CACHE[L] = _build(L)
    return _BUILD_CACHE[L]


def _masks(aquad=False):
    """0/1 single-tile masks [j 128, qi 128 | A] in the transposed-scores
    orientation. Region B (cols 0:128): keys in the query's own x-tile —
    strict causal p < qi. Region A: the preceding x-tile — only its second
    chunk and only for the first-chunk queries (qi < 64). With aquad the
    region-A block is the [64, 64] fully-allowed quadrant (mask = 1 on
    partitions 0:64). m01z is the no-previous-chunk variant for global
    chunk 0 (region A fully zero)."""
    import ml_dtypes

    p = np.arange(128)[:, None]
    q = np.arange(128)[None, :]
    mb_ = (p < q).astype(np.float32)
    if aquad:
        ma_ = np.zeros((128, 64), np.float32)
        ma_[64:128, :] = 1.0
    else:
        ma_ = ((p >= 64) & (q < 64)).astype(np.float32)
    m01 = np.concatenate([mb_, ma_], axis=1).astype(ml_dtypes.bfloat16)
    m01z = np.concatenate([mb_, np.zeros_like(ma_)],
                          axis=1).astype(ml_dtypes.bfloat16)
    return m01, m01z


def _make_in_maps(states, Wq, Wk, Wv, aquad=False, fp8_proj=False):
    import ml_dtypes

    bf16 = ml_dtypes.bfloat16
    f8 = ml_dtypes.float8_e4m3
    states = np.ascontiguousarray(np.asarray(states, dtype=np.float32))
    B, S = states.shape[0], states.shape[1]
    x = states.reshape(B, S, _D)
    L = S // 2
    NX = L + _HALO
    Wq = np.asarray(Wq, dtype=np.float32)
    Wk = np.asarray(Wk, dtype=np.float32)
    Wv = np.asarray(Wv, dtype=np.float32)
    # scores = q.k^T = x (Wq^T Wk) x^T — fold both weights into one matrix
    mw_f = np.ascontiguousarray(Wq.T @ Wk)
    wvt_f = np.ascontiguousarray(Wv.T)
    if fp8_proj:
        mw = (mw_f * _W8SCALE).astype(f8).reshape(256, 2, _D)
        wvt = (wvt_f * _W8SCALE).astype(f8).reshape(256, 2, _D)
    else:
        mw = mw_f.astype(bf16)
        wvt = wvt_f.astype(bf16)
    m01, m01z = _masks(aquad)
    in_maps = []
    for core in range(2 * B):
        b, h = core // 2, core % 2
        xp = np.zeros((NX, _D), dtype=np.float32)
        lo = h * L - _HALO
        if lo < 0:
            xp[_HALO:] = x[b, 0:L]
        else:
            xp[:] = x[b, lo:lo + NX]
        xT = np.ascontiguousarray(xp.T)
        m = {
            "xT": xT.astype(bf16),
            "mw": mw, "wvt": wvt,
            "m01": m01, "m01z": m01z if h == 0 else m01,
        }
        if fp8_proj:
            m["x8"] = xT.astype(f8).reshape(256, 2, NX)
        in_maps.append(m)
    return in_maps


def _assemble(results, B, S):
    L = S // 2
    out = np.empty((B, S, _D), dtype=np.float32)
    for core, res in enumerate(results):
        b, h = core // 2, core % 2
        out[b, h * L:(h + 1) * L] = res["out"].astype(np.float32)
    out[:, 0, :] = 0.0
    return out.reshape(B, S, _D // 2, 2)


def run(states, Wq, Wk, Wv, trace=False):
    """Run on 8 NeuronCores; returns (output, BassKernelResults)."""
    from concourse.bass_utils import run_bass_kernel_spmd

    states = np.asarray(states)
    B, S = states.shape[0], states.shape[1]
    assert B == 4 and S % 128 == 0
    nc = _get_nc(S // 2)
    in_maps = _make_in_maps(states, Wq, Wk, Wv)
    try:
        res = run_bass_kernel_spmd(nc, in_maps, core_ids=list(range(8)), trace=trace)
    except ModuleNotFoundError:
        # axon NTFF hook unavailable in this container — run untraced
        res = run_bass_kernel_spmd(nc, in_maps, core_ids=list(range(8)), trace=False)
    return _assemble(res.results, B, S), res


def kernel(states, Wq, Wk, Wv):
    out, _ = run(states, Wq, Wk, Wv, trace=False)
    return out
